# revision 1
# baseline (speedup 1.0000x reference)
"""Boundary-smoothing masked-BCE kernel for Trainium2 (8 NeuronCores).

Math (reference, SB_SIZE=1, SB_EPSILON=0.1):
    P = (target==1), M = (mask==1)
    cnt = 4-neighbor sum of M (s1 +/-1, s2 +/-1), add = same of P
    b2l = P - 0.025*P*cnt + 0.025*M*add
    out = sum(M * (softplus(x) - x*b2l)) / sum(M)

Decomposition used here (nbr == self-adjoint 4-neighbor sum):
    num = SUM_all softplus(x*M) - kappa*(N - SumM)
          - SUM xm*P - 0.025*SUM xm*nbr(P) + 0.025*SUM (xm*P)*nbr(M)
    where xm = x*M and kappa is the device's own value of softplus(0)
    (probed on-chip, so the masked-out lanes cancel exactly).

Per core: 2 batches.  Layout: s1 split even/odd over 128 partitions so the
s1-stencil is a banded 128x128 matmul contracting the opposite parity;
s2-stencil is two shifted-identity matmuls accumulating into the same PSUM.
Host pads target/mask with one zero s2-column on each side so halo DMAs are
always in-bounds (no margin memsets).
"""
import sys

sys.path.insert(0, "/opt/trn_rl_repo")

import numpy as np
import ml_dtypes

import concourse.bass as bass
import concourse.bacc as bacc
import concourse.tile as tile
import concourse.mybir as mybir
from concourse.bass_utils import run_bass_kernel_spmd

bf16 = mybir.dt.bfloat16
f32 = mybir.dt.float32
i32 = mybir.dt.int32

B, S, L = 16, 256, 24
NCORES = 8
BLOC = B // NCORES            # 2 batches per core
P = 128                       # partitions = s1 parity rows
F = S * L                     # 6144 free cols (s2, l)
HALF = F // 2                 # 3072
MG = L                        # 24-col halo = one s2 step
HW_COLS = HALF + 2 * MG       # 3120 (halo-padded strip width)
PIECE = 1024                  # psum piece (2 banks)
SUB = 512                     # matmul free chunk (1 bank)
N_CORE = BLOC * S * S * L     # elements per core

MULT = mybir.AluOpType.mult
ADD = mybir.AluOpType.add
AX = mybir.AxisListType.X
AF = mybir.ActivationFunctionType


def _build_wconst():
    we = np.eye(P) + np.diag(np.ones(P - 1), 1)    # out_e[m] = O[m-1]+O[m]
    wo = np.eye(P) + np.diag(np.ones(P - 1), -1)   # out_o[m] = E[m]+E[m+1]
    ident = np.eye(P)
    w = np.zeros((P, 392), dtype=np.float32)
    w[:, 0:128] = we
    w[:, 128:256] = wo
    w[:, 256:384] = ident
    w[:, 384] = 1.0                                # ones column
    return w.astype(ml_dtypes.bfloat16)


def _build_bass():
    nc = bacc.Bacc("TRN2", target_bir_lowering=False)
    pred = nc.declare_dram_parameter("predict", [BLOC, S, S, L], f32, isOutput=False)
    targ = nc.declare_dram_parameter("target", [BLOC, S, S + 2, L], f32, isOutput=False)
    mask = nc.declare_dram_parameter("mask", [BLOC, S, S + 2, L], i32, isOutput=False)
    wcon = nc.declare_dram_parameter("wconst", [P, 392], bf16, isOutput=False)
    out = nc.declare_dram_parameter("out", [P, 16], f32, isOutput=True)

    xr = pred.rearrange("b (s1 two) s2 l -> b two s1 (s2 l)", two=2)   # [2,2,128,6144]
    tr = targ.rearrange("b (s1 two) s2 l -> b two s1 (s2 l)", two=2)   # [2,2,128,6192]
    mr = mask.rearrange("b (s1 two) s2 l -> b two s1 (s2 l)", two=2)

    with tile.TileContext(nc) as tc:
        _body(tc, xr, tr, mr, wcon, out)
    nc.compile()
    _dedup_act_table_loads(nc)
    return nc


def _dedup_act_table_loads(nc):
    # All our ACT funcs (Exp, Ln, Copy) live together in
    # natural_log_exp_and_others (canonical set id 6).  bacc's per-function
    # canonical choice alternates exp_and_others(0) / natural_log(5), paying
    # a ~2.7us table DMA per switch.  The emitted loads carry no semaphores,
    # so: point the first one at set 6 and drop the rest.
    from concourse.hw_specs import get_activation_tables
    names = list(get_activation_tables("gen3").keys())
    target = names.index("natural_log_exp_and_others")
    for bb in nc.main_func.blocks:
        keep = []
        first = True
        for ins in bb.instructions:
            if type(ins).__name__ == "InstLoadActFuncSet":
                si = ins.sync_info
                if si is not None and (si.on_wait or si.on_update):
                    keep.append(ins)
                    continue
                if first:
                    ins.act_func_set_id = target
                    keep.append(ins)
                    first = False
                continue
            keep.append(ins)
        if len(keep) != len(bb.instructions):
            bb.instructions = keep


def _body(tc, xr, tr, mr, wcon, out):
    nc = tc.nc
    import contextlib
    ctx = contextlib.ExitStack()
    with ctx:
        const = ctx.enter_context(tc.tile_pool(name="const", bufs=1))
        accp = ctx.enter_context(tc.tile_pool(name="accp", bufs=1))
        inx = ctx.enter_context(tc.tile_pool(name="inx", bufs=3))
        inp = ctx.enter_context(tc.tile_pool(name="inp", bufs=3))
        inm = ctx.enter_context(tc.tile_pool(name="inm", bufs=3))
        mid = ctx.enter_context(tc.tile_pool(name="mid", bufs=4))
        nbp = ctx.enter_context(tc.tile_pool(name="nbp", bufs=4))
        scr = ctx.enter_context(tc.tile_pool(name="scr", bufs=2))
        zp = ctx.enter_context(tc.tile_pool(name="zp", bufs=4))
        pstp = ctx.enter_context(tc.tile_pool(name="pstp", bufs=2, space="PSUM"))
        psrow = ctx.enter_context(tc.tile_pool(name="psrow", bufs=1, space="PSUM"))

        wt = const.tile([P, 392], bf16)
        nc.sync.dma_start(out=wt, in_=wcon[:, :])
        W_E, W_O, IDN, ONE = 0, 128, 256, 384

        tch = const.tile([P, 32], bf16)      # DVE touch scratch (rotating cols)
        accSP = accp.tile([P, 8], f32)       # per-iteration softplus row sums
        outt = accp.tile([P, 16], f32)
        rowY = psrow.tile([1, SUB], f32)
        rowZ1 = psrow.tile([1, SUB], f32)
        rowZ2 = psrow.tile([1, SUB], f32)
        rowM = psrow.tile([1, SUB], f32)
        row_started = {}

        nc.vector.memset(outt, 0.0)
        nc.vector.memset(accSP, 0.0)

        # kappa probe: softplus(0) through the exact same Exp/Ln pipeline.
        kz = const.tile([1, 8], bf16)
        ke = const.tile([1, 8], f32)
        ks = const.tile([1, 8], bf16)
        kacc = const.tile([1, 1], f32)
        nc.vector.memset(kz, 0.0)
        nc.scalar.activation(ke, kz, AF.Exp)
        nc.scalar.activation(ks, ke, AF.Ln, bias=1.0, accum_out=kacc[0:1, 0:1])
        ktch = const.tile([1, 1], bf16)
        nc.vector.tensor_copy(ktch, ks[0:1, 0:1])
        nc.vector.tensor_copy(outt[0:1, 3:4], kacc[0:1, 0:1])

        tcol = [0]

        def dtouch(src_ap):
            c = tcol[0] % 32
            tcol[0] += 1
            nc.vector.tensor_copy(tch[:, c:c + 1], src_ap)

        def row_mm(rowt, rhs_ap):
            st = id(rowt) not in row_started
            row_started[id(rowt)] = True
            nc.tensor.matmul(rowt[0:1, :], lhsT=wt[:, ONE:ONE + 1],
                             rhs=rhs_ap, start=st, stop=False)

        it8 = 0
        for ib in range(BLOC):
            for half in range(2):
                xb = [inx.tile([P, HALF], bf16, tag="xb", name="xb") for _ in range(2)]
                pb = [inp.tile([P, HW_COLS], bf16, tag="pb", name="pb") for _ in range(2)]
                mb = [inm.tile([P, HW_COLS], bf16, tag="mb", name="mb") for _ in range(2)]
                for par in range(2):
                    nc.gpsimd.dma_start(
                        out=xb[par], in_=xr[ib, par][:, half * HALF:(half + 1) * HALF])
                    nc.gpsimd.dma_start(
                        out=pb[par], in_=tr[ib, par][:, half * HALF:half * HALF + HW_COLS])
                    nc.gpsimd.dma_start(
                        out=mb[par], in_=mr[ib, par][:, half * HALF:half * HALF + HW_COLS])

                # absorb the six DMA ticks one at a time (DVE), then PE
                for par in range(2):
                    dtouch(xb[par][:, 0:1])
                    dtouch(pb[par][:, 0:1])
                    dtouch(mb[par][:, 0:1])

                xm = [None, None]
                yb = [None, None]
                for par in range(2):
                    xm[par] = mid.tile([P, HALF], bf16, tag="xm", name="xm")
                    nc.vector.tensor_tensor(
                        xm[par], mb[par][:, MG:MG + HALF], xb[par], op=MULT)
                    e = scr.tile([P, HALF], f32)
                    nc.scalar.activation(e, xm[par], AF.Exp)
                    sps = scr.tile([P, HALF], bf16)
                    nc.scalar.activation(sps, e, AF.Ln, bias=1.0,
                                         accum_out=accSP[:, it8 + par:it8 + par + 1])
                    yb[par] = mid.tile([P, HALF], bf16, tag="yb", name="yb")
                    nc.vector.tensor_tensor(
                        yb[par], xm[par], pb[par][:, MG:MG + HALF], op=MULT)

                for par in range(2):
                    opp = 1 - par
                    z1s, z2s = [], []
                    wband = wt[:, (W_E if par == 0 else W_O):(W_E if par == 0 else W_O) + 128]
                    # ---- P stream: nbP -> z1 = xm * nbP (ACT drains) ----
                    for pc in range(3):
                        ps = pstp.tile([P, PIECE], f32)
                        d0 = pc * PIECE
                        for s in range(2):
                            c = MG + d0 + s * SUB
                            nc.tensor.matmul(ps[:, s * SUB:(s + 1) * SUB],
                                             lhsT=wband, rhs=pb[opp][:, c:c + SUB],
                                             start=True, stop=False)
                        for s in range(2):
                            c = MG + d0 + s * SUB
                            nc.tensor.matmul(ps[:, s * SUB:(s + 1) * SUB],
                                             lhsT=wt[:, IDN:IDN + 128],
                                             rhs=pb[par][:, c - MG:c - MG + SUB],
                                             start=False, stop=False)
                            nc.tensor.matmul(ps[:, s * SUB:(s + 1) * SUB],
                                             lhsT=wt[:, IDN:IDN + 128],
                                             rhs=pb[par][:, c + MG:c + MG + SUB],
                                             start=False, stop=True)
                        nb = nbp.tile([P, PIECE], bf16)
                        nc.scalar.activation(nb, ps, AF.Copy)
                        dtouch(nb[:, 0:1])             # DVE observes ACT drain tick
                        z1 = zp.tile([P, PIECE], bf16, tag="z1", name="z1")
                        nc.vector.tensor_tensor(z1, xm[par][:, d0:d0 + PIECE], nb, op=MULT)
                        z1s.append(z1)
                    # ---- M stream: nbM -> z2 = yb * nbM (DVE drains) ----
                    for pc in range(3):
                        ps = pstp.tile([P, PIECE], f32)
                        d0 = pc * PIECE
                        for s in range(2):
                            c = MG + d0 + s * SUB
                            nc.tensor.matmul(ps[:, s * SUB:(s + 1) * SUB],
                                             lhsT=wband, rhs=mb[opp][:, c:c + SUB],
                                             start=True, stop=False)
                        for s in range(2):
                            c = MG + d0 + s * SUB
                            nc.tensor.matmul(ps[:, s * SUB:(s + 1) * SUB],
                                             lhsT=wt[:, IDN:IDN + 128],
                                             rhs=mb[par][:, c - MG:c - MG + SUB],
                                             start=False, stop=False)
                            nc.tensor.matmul(ps[:, s * SUB:(s + 1) * SUB],
                                             lhsT=wt[:, IDN:IDN + 128],
                                             rhs=mb[par][:, c + MG:c + MG + SUB],
                                             start=False, stop=True)
                        nb2 = nbp.tile([P, PIECE], bf16)
                        nc.vector.tensor_copy(nb2, ps)
                        z2 = zp.tile([P, PIECE], bf16, tag="z2", name="z2")
                        nc.vector.tensor_tensor(z2, yb[par][:, d0:d0 + PIECE], nb2, op=MULT)
                        z2s.append(z2)
                    # batched rows: single ones-weight load per parity
                    for z1 in z1s:
                        for s in range(2):
                            row_mm(rowZ1, z1[:, s * SUB:(s + 1) * SUB])
                    for z2 in z2s:
                        for s in range(2):
                            row_mm(rowZ2, z2[:, s * SUB:(s + 1) * SUB])
                    # fold Y and M 3072->1536 on DVE (exact for 0/1 mask sums)
                    yfold = zp.tile([P, HALF // 2], bf16, tag="yfold", name="yfold")
                    nc.vector.tensor_tensor(yfold, yb[par][:, 0:HALF // 2],
                                            yb[par][:, HALF // 2:HALF], op=ADD)
                    mfold = zp.tile([P, HALF // 2], bf16, tag="mfold", name="mfold")
                    nc.vector.tensor_tensor(mfold, mb[par][:, MG:MG + HALF // 2],
                                            mb[par][:, MG + HALF // 2:MG + HALF], op=ADD)
                    for s in range(3):
                        row_mm(rowY, yfold[:, s * SUB:(s + 1) * SUB])
                    for s in range(3):
                        row_mm(rowM, mfold[:, s * SUB:(s + 1) * SUB])
                it8 += 2

        # finals
        dtouch(accSP[:, 0:1])                       # DVE observes last ACT tick
        nc.vector.tensor_reduce(outt[:, 0:1], accSP, axis=AX, op=ADD)
        nc.vector.tensor_reduce(outt[0:1, 4:5], rowY, axis=AX, op=ADD)
        nc.vector.tensor_reduce(outt[0:1, 5:6], rowZ1, axis=AX, op=ADD)
        nc.vector.tensor_reduce(outt[0:1, 6:7], rowZ2, axis=AX, op=ADD)
        nc.vector.tensor_reduce(outt[0:1, 7:8], rowM, axis=AX, op=ADD)
        nc.sync.dma_start(out=out[:, :], in_=outt)


_BASS_CACHE = {}


def _get_bass():
    if "nc" not in _BASS_CACHE:
        _BASS_CACHE["nc"] = _build_bass()
        _BASS_CACHE["wconst"] = _build_wconst()
    return _BASS_CACHE["nc"], _BASS_CACHE["wconst"]


def kernel(predict, target, mask):
    predict = np.ascontiguousarray(np.asarray(predict, dtype=np.float32))
    target = np.asarray(target, dtype=np.float32)
    mask = np.asarray(mask, dtype=np.int32)

    tpad = np.zeros((B, S, S + 2, L), dtype=np.float32)
    tpad[:, :, 1:S + 1, :] = target
    mpad = np.zeros((B, S, S + 2, L), dtype=np.int32)
    mpad[:, :, 1:S + 1, :] = mask

    nc, wconst = _get_bass()
    in_maps = []
    for c in range(NCORES):
        b0 = c * BLOC
        in_maps.append({
            "predict": np.ascontiguousarray(predict[b0:b0 + BLOC]),
            "target": np.ascontiguousarray(tpad[b0:b0 + BLOC]),
            "mask": np.ascontiguousarray(mpad[b0:b0 + BLOC]),
            "wconst": wconst,
        })
    res = run_bass_kernel_spmd(nc, in_maps, list(range(NCORES)))

    num = 0.0
    den = 0.0
    for c in range(NCORES):
        o = res.results[c]["out"].astype(np.float64)
        sum_sp = o[:, 0].sum()
        kappa = o[0, 3] / 8.0
        sum_y = o[0, 4]
        sum_z1 = o[0, 5]
        sum_z2 = o[0, 6]
        sum_m = o[0, 7]
        num += (sum_sp - kappa * (N_CORE - sum_m)
                - sum_y - 0.025 * sum_z1 + 0.025 * sum_z2)
        den += sum_m
    return np.float32(num / den)

